# revision 15
# baseline (speedup 1.0000x reference)
"""Trainium2 Bass kernel for nn_CrossAttention_249108103802.

Math (per batch b, one NeuronCore; 8 cores data-parallel over B=8):
  q_s   = heads(x_s)                   (column slices of x_s)
  k,v   = x_s @ W_s  split per head    -> never materialized; instead:
  ctx_s = softmax_d(scale * k^T v)     via Gram trick:
          k_h^T v_h = Wk_h^T (x^T x) Wv_h
  o1    = q1 @ blockdiag(ctx2), o2 = q2 @ blockdiag(ctx1)

Precision: bf16 operands on the PE with fp32 PSUM accumulation. The Gram
matrix is split G = Gc + mu*I (mu = N) so Gc fits bf16; the mu*Wv term is
re-added inside the A = G @ Wv matmul via a bf16 mu*I stationary. A gets a
hi/lo bf16 split before ctp = A^T Wk. SCALE is folded into the Wk cast, and
softmax is stabilized with a per-row max (negated reduce as the exp bias).

Schedule: x1 streams in (SWDGE fp32->bf16 cast) feeding Gram1 + PE
transposes immediately; W loads ride the HWDGE queue concurrently; then
Gram2, ctx1, A2/ctp2, out2 (overlapping softmax2), out1. Out-stage matmuls
rotate 4 PSUM banks with evacuation split across scalar/vector.
"""
import sys

sys.path.insert(0, "/opt/trn_rl_repo")

import numpy as np

import concourse.bass as bass
import concourse.mybir as mybir
import concourse.tile as tile
from concourse import bacc
from concourse.bass_utils import run_bass_kernel_spmd
from concourse.masks import make_identity

B, N, C, H = 8, 4096, 512, 8
HD = C // H                    # 64
SCALE = HD ** -0.5             # 1/8
MU = float(N)                  # expected Gram diagonal
NT = N // 128                  # 32 row tiles
CB = C // 128                  # 4 feature blocks
BF = mybir.dt.bfloat16
F32 = mybir.dt.float32
AF = mybir.ActivationFunctionType
AX = mybir.AxisListType


def build():
    nc = bacc.Bacc("TRN2", target_bir_lowering=False, debug=False, num_devices=8)
    x_d = [nc.declare_dram_parameter("x1", [N, C], F32, isOutput=False),
           nc.declare_dram_parameter("x2", [N, C], F32, isOutput=False)]
    w_d = [nc.declare_dram_parameter("W_kv1", [C, 2 * C], F32, isOutput=False),
           nc.declare_dram_parameter("W_kv2", [C, 2 * C], F32, isOutput=False)]
    o_d = [nc.declare_dram_parameter("o1", [N, C], BF, isOutput=True),
           nc.declare_dram_parameter("o2", [N, C], BF, isOutput=True)]

    with tile.TileContext(nc) as tc:
        with (
            tc.tile_pool(name="const", bufs=1) as constp,
            tc.tile_pool(name="wf", bufs=2) as wfp,
            tc.tile_pool(name="w", bufs=1) as wp,
            tc.tile_pool(name="x", bufs=4) as xp,
            tc.tile_pool(name="xt", bufs=1) as xtp,
            tc.tile_pool(name="g", bufs=1) as gp_,
            tc.tile_pool(name="a", bufs=1) as ap_,
            tc.tile_pool(name="ctx", bufs=2) as cxp,
            tc.tile_pool(name="osb", bufs=3) as osp,
            tc.tile_pool(name="ps_g", bufs=1, space="PSUM") as psg,
            tc.tile_pool(name="ps_t", bufs=2, space="PSUM") as pst,
            tc.tile_pool(name="ps_o", bufs=2, space="PSUM") as pso,
        ):
            ident = constp.tile([128, 128], BF, tag="ident")
            make_identity(nc, ident[:])
            # mu*I tiles are created later (after the x1 DMA issues) so the
            # gpsimd stream reaches the first chunk descriptor ASAP.
            muI = constp.tile([128, 128], F32, tag="muI")
            muIb = constp.tile([128, 128], BF, tag="muIb")

            def make_mu_tiles():
                for t_ in (muI, muIb):
                    nc.gpsimd.memset(t_[:], 0.0)
                    nc.gpsimd.affine_select(
                        out=t_[:], in_=t_[:],
                        compare_op=mybir.AluOpType.not_equal, fill=MU,
                        base=0, pattern=[[-1, 128]], channel_multiplier=1,
                    )

            def copy_alt(i, out, in_):
                if i % 2 == 0:
                    nc.scalar.copy(out, in_)
                else:
                    nc.vector.tensor_copy(out, in_)

            xts, gsbs, gtrs, lows, cbds = [], [], [], [], []

            # ---- load x_s, Gram accumulation, and PE transposes ----
            def gram_stage(s, after_dma=None):
                gps = []
                for m in range(CB):
                    gt_ = psg.tile([128, 512], F32, tag=f"g{m}",
                                   name=f"gp{m}_{s}")
                    gps.append(gt_)
                xt = xtp.tile([128, CB, N], BF, tag=f"xt{s}")
                xts.append(xt)
                chunks = []
                for r in range(NT // 4):
                    # fp32 DRAM -> bf16 SBUF (SWDGE cast), 1 MiB fp32 chunk.
                    xc = xp.tile([128, 4, C], BF, tag="xc", name=f"xc{s}_{r}")
                    src = x_d[s][512 * r:512 * (r + 1), :].rearrange(
                        "(t p) c -> p t c", p=128)
                    nc.gpsimd.dma_start(out=xc[:], in_=src)
                    chunks.append(xc)
                    if r == 0 and after_dma is not None:
                        after_dma()
                for r in range(NT // 4):
                    xc = chunks[r]
                    for tt_ in range(4):
                        t = 4 * r + tt_
                        for m in range(CB):
                            nc.tensor.matmul(
                                gps[m][:, :C - 128 * m],
                                lhsT=xc[:, tt_, 128 * m:128 * (m + 1)],
                                rhs=xc[:, tt_, 128 * m:],
                                start=(t == 0), stop=(t == NT - 1),
                            )
                        tp4 = pst.tile([128, CB, 128], BF, tag="tps",
                                       name=f"tp4_{s}_{t}")
                        for cb in range(CB):
                            nc.tensor.transpose(
                                tp4[:, cb, :], xc[:, tt_, 128 * cb:128 * (cb + 1)],
                                ident[:])
                        copy_alt(t, xt[:, :, 128 * t:128 * (t + 1)], tp4[:])

                # Gc = G - mu*I in bf16; gsb[:, a, f] = G[128a+p, f]
                gsb = gp_.tile([128, CB, C], BF, tag=f"g{s}")
                for m in range(CB):
                    nc.vector.tensor_sub(
                        gsb[:, m, 128 * m:128 * (m + 1)],
                        gps[m][:, 0:128], muI[:])
                    if m < CB - 1:
                        copy_alt(m, gsb[:, m, 128 * (m + 1):],
                                 gps[m][:, 128:C - 128 * m])
                # lower-triangle tiles by PE transpose
                low = {}
                gtr = gp_.tile([128, 6, 128], BF, tag=f"gt{s}")
                idx = 0
                for a2 in range(CB):
                    for b2 in range(a2):
                        tp = pst.tile([128, 128], BF, tag="tps")
                        nc.tensor.transpose(
                            tp[:], gsb[:, b2, 128 * a2:128 * (a2 + 1)], ident[:])
                        nc.scalar.copy(gtr[:, idx, :], tp[:])
                        low[(a2, b2)] = idx
                        idx += 1
                gsbs.append(gsb)
                gtrs.append(gtr)
                lows.append(low)

            # ---- weights: load + cast (k-half pre-scaled by SCALE) ----
            whi = []

            wfs = []

            def weight_load(s):
                # SWDGE queue, ordered behind the x chunks so the W bytes
                # never contend with the x-load ramp.
                wf = wfp.tile([128, CB, 2 * C], F32, tag="wf",
                              name=f"wf{s}")
                src = w_d[s][:, :].rearrange("(a p) m -> p a m", p=128)
                nc.gpsimd.dma_start(out=wf[:], in_=src)
                wfs.append(wf)
                wh = wp.tile([128, CB, 2 * C], BF, tag=f"w{s}",
                             name=f"wh{s}")
                whi.append(wh)

            def weight_cast(s):
                # v-half plain cast on scalar, k-half scaled (SCALE = 2^-3,
                # exact in bf16) on vector; scheduled into engine slack.
                wf, wh = wfs[s], whi[s]
                for a in range(CB):
                    nc.scalar.copy(wh[:, a, C:], wf[:, a, C:])
                    nc.vector.tensor_scalar_mul(
                        wh[:, a, 0:C], wf[:, a, 0:C], SCALE)

            # ---- ctx_s: A = G @ Wv (mu folded in), ctp = A^T Wk_scaled ----
            def ctx_mm_stage(s):
                gsb, gtr, low = gsbs[s], gtrs[s], lows[s]

                def g_tile(a2, b2):
                    if b2 >= a2:
                        return gsb[:, a2, 128 * b2:128 * (b2 + 1)]
                    return gtr[:, low[(a2, b2)], :]

                wh = whi[s]
                ahi = ap_.tile([128, CB, C], BF, tag="ah", name=f"ah{s}")
                alo = ap_.tile([128, CB, C], BF, tag="al", name=f"al{s}")
                for b2 in range(CB):
                    apx = pso.tile([128, C], F32, tag="pbig", name=f"apx{s}_{b2}")
                    for a2 in range(CB):
                        nc.tensor.matmul(
                            apx[:], lhsT=g_tile(a2, b2), rhs=wh[:, a2, C:],
                            start=(a2 == 0), stop=False)
                    # += mu * Wv rows of block b2  (G = Gc + mu*I)
                    nc.tensor.matmul(
                        apx[:], lhsT=muIb[:], rhs=wh[:, b2, C:],
                        start=False, stop=True)
                    # hi/lo split; hi halves on scalar & vector in parallel
                    hm = C // 2
                    nc.scalar.copy(ahi[:, b2, 0:hm], apx[:, 0:hm])
                    nc.vector.tensor_copy(ahi[:, b2, hm:], apx[:, hm:])
                    nc.vector.tensor_sub(alo[:, b2, :], apx[:], ahi[:, b2, :])

                # ctp[e, d] per head = (A^T Wk_scaled), hi + lo
                ctp = pso.tile([64, C], F32, tag="pbig", name=f"ctp{s}")
                for h in range(H):
                    sl = slice(64 * h, 64 * (h + 1))
                    for a2 in range(CB):
                        nc.tensor.matmul(
                            ctp[:, sl], lhsT=ahi[:, a2, sl], rhs=wh[:, a2, sl],
                            start=(a2 == 0), stop=False)
                    for a2 in range(CB):
                        nc.tensor.matmul(
                            ctp[:, sl], lhsT=alo[:, a2, sl], rhs=wh[:, a2, sl],
                            start=False, stop=(a2 == CB - 1))
                return ctp

            # ---- softmax over d (free axis) + block-diag ctx tiles ----
            ctxtss = []

            def softmax_pre(s, ctp):
                esb = cxp.tile([64, C], F32, tag="esb")
                ssum = cxp.tile([64, H], F32, tag="ssum")
                rsum = cxp.tile([64, H], F32, tag="rsum")
                nmax = cxp.tile([64, H], F32, tag="nmax")
                ctxts = cxp.tile([64, C], BF, tag="ctxts")
                for h in range(H):
                    sl = slice(64 * h, 64 * (h + 1))
                    nc.vector.reduce_max(
                        nmax[:, h:h + 1], ctp[:, sl], axis=AX.X, negate=True)
                for h in range(H):
                    sl = slice(64 * h, 64 * (h + 1))
                    nc.scalar.activation(
                        esb[:, sl], ctp[:, sl], AF.Exp,
                        bias=nmax[:, h:h + 1], accum_out=ssum[:, h:h + 1])
                nc.vector.reciprocal(rsum[:], ssum[:])
                for h in range(H):
                    sl = slice(64 * h, 64 * (h + 1))
                    nc.vector.tensor_scalar_mul(
                        ctxts[:, sl], esb[:, sl], rsum[:, h:h + 1])
                ctxtss.append(ctxts)

            def softmax_cbd(s):
                ctxts = ctxtss[s]
                # 2 heads per PE transpose -> natural ctx block-diag tiles
                cbd = cxp.tile([128, CB, 128], BF, tag=f"cbd{s}")
                nc.vector.memset(cbd[:], 0.0)
                for t2 in range(CB):
                    tp = pst.tile([128, 128], BF, tag="tps")
                    nc.tensor.transpose(
                        tp[:, 0:64], ctxts[:, 128 * t2:128 * (t2 + 1)],
                        ident[0:64, 0:64])
                    nc.scalar.copy(cbd[0:64, t2, 0:64], tp[0:64, 0:64])
                    nc.scalar.copy(cbd[64:128, t2, 64:128], tp[64:128, 0:64])
                cbds.append(cbd)

            # ---- o_s = x_s @ blockdiag(ctx_other), 4-bank rotation ----
            def out_stage(s):
                xt, cbd = xts[s], cbds[1 - s]
                for r in range(NT // 4):
                    ob = osp.tile([128, 4, C], BF, tag="ob")
                    for pp in range(2):       # tile pairs within the group
                        ops = []
                        for q in range(2):
                            t = 4 * r + 2 * pp + q
                            bank = 2 * pp + q
                            op = psg.tile([128, 512], F32, tag=f"g{bank}",
                                          name=f"op{s}_{t}")
                            ops.append(op)
                        for cb in range(CB):  # alternate banks per MM
                            for q in range(2):
                                t = 4 * r + 2 * pp + q
                                nc.tensor.matmul(
                                    ops[q][:, 128 * cb:128 * (cb + 1)],
                                    lhsT=xt[:, cb, 128 * t:128 * (t + 1)],
                                    rhs=cbd[:, cb, :],
                                    start=True, stop=True)
                        for q in range(2):
                            copy_alt(q, ob[:, 2 * pp + q, :], ops[q][:])
                    dst = o_d[s][512 * r:512 * (r + 1), :].rearrange(
                        "(t p) c -> p t c", p=128)
                    nc.sync.dma_start(out=dst, in_=ob[:])

            # SWDGE queue order: x1 chunks, W1, x2 chunks, W2.
            gram_stage(0, after_dma=make_mu_tiles)   # x1 load, G1, xT1
            weight_load(0)
            weight_cast(0)
            gram_stage(1)              # x2 load, G2, xT2
            weight_load(1)
            weight_cast(1)
            ctp1 = ctx_mm_stage(0)
            softmax_pre(0, ctp1)
            ctp2 = ctx_mm_stage(1)
            softmax_cbd(0)             # -> cbds[0]
            softmax_pre(1, ctp2)       # overlaps out2 below
            out_stage(1)               # o2 = x2 @ Cbd1
            softmax_cbd(1)             # -> cbds[1]
            out_stage(0)               # o1 = x1 @ Cbd2
    nc.compile()
    return nc


_NC = None


def kernel(x1, x2, W_kv1, W_kv2):
    global _NC
    if _NC is None:
        _NC = build()
    x1 = np.ascontiguousarray(x1, dtype=np.float32)
    x2 = np.ascontiguousarray(x2, dtype=np.float32)
    W1 = np.ascontiguousarray(W_kv1, dtype=np.float32)
    W2 = np.ascontiguousarray(W_kv2, dtype=np.float32)
    in_maps = [
        {"x1": x1[b], "x2": x2[b], "W_kv1": W1, "W_kv2": W2} for b in range(B)
    ]
    res = run_bass_kernel_spmd(_NC, in_maps, core_ids=list(range(B)))
    o1 = np.stack([res.results[b]["o1"].astype(np.float32) for b in range(B)])
    o2 = np.stack([res.results[b]["o2"].astype(np.float32) for b in range(B)])
    return o1, o2


# revision 24
# speedup vs baseline: 1.0124x; 1.0124x over previous
"""Trainium2 Bass kernel for nn_CrossAttention_249108103802.

Math (per batch b, one NeuronCore; 8 cores data-parallel over B=8):
  q_s   = heads(x_s)                   (column slices of x_s)
  k,v   = x_s @ W_s  split per head    -> never materialized; instead:
  ctx_s = softmax_d(scale * k^T v)     via Gram trick:
          k_h^T v_h = Wk_h^T (x^T x) Wv_h
  o1    = q1 @ blockdiag(ctx2), o2 = q2 @ blockdiag(ctx1)

Precision: bf16 operands on the PE with fp32 PSUM accumulation. The Gram
matrix is split G = Gc + mu*I (mu = N) so Gc fits bf16; the mu*Wv term is
re-added inside the A = G @ Wv matmul via a bf16 mu*I stationary. A gets a
hi/lo bf16 split before ctp = A^T Wk. SCALE is folded into the Wk cast, and
softmax is stabilized with a per-row max (negated reduce as the exp bias).

Schedule: x1 streams in (SWDGE fp32->bf16 cast) feeding Gram1 + PE
transposes immediately; W loads ride the HWDGE queue concurrently; then
Gram2, ctx1, A2/ctp2, out2 (overlapping softmax2), out1. Out-stage matmuls
rotate 4 PSUM banks with evacuation split across scalar/vector.
"""
import sys

sys.path.insert(0, "/opt/trn_rl_repo")

import numpy as np

import concourse.bass as bass
import concourse.mybir as mybir
import concourse.tile as tile
from concourse import bacc
from concourse.bass_utils import run_bass_kernel_spmd
from concourse.masks import make_identity

B, N, C, H = 8, 4096, 512, 8
HD = C // H                    # 64
SCALE = HD ** -0.5             # 1/8
MU = float(N)                  # expected Gram diagonal
NT = N // 128                  # 32 row tiles
CB = C // 128                  # 4 feature blocks
BF = mybir.dt.bfloat16
F32 = mybir.dt.float32
AF = mybir.ActivationFunctionType
AX = mybir.AxisListType


def build():
    nc = bacc.Bacc("TRN2", target_bir_lowering=False, debug=False, num_devices=8)
    x_d = [nc.declare_dram_parameter("x1", [N, C], F32, isOutput=False),
           nc.declare_dram_parameter("x2", [N, C], F32, isOutput=False)]
    w_d = [nc.declare_dram_parameter("W_kv1", [C, 2 * C], F32, isOutput=False),
           nc.declare_dram_parameter("W_kv2", [C, 2 * C], F32, isOutput=False)]
    o_d = [nc.declare_dram_parameter("o1", [N, C], BF, isOutput=True),
           nc.declare_dram_parameter("o2", [N, C], BF, isOutput=True)]

    with tile.TileContext(nc) as tc:
        with (
            tc.tile_pool(name="const", bufs=1) as constp,
            tc.tile_pool(name="wf", bufs=2) as wfp,
            tc.tile_pool(name="w", bufs=1) as wp,
            tc.tile_pool(name="x", bufs=4) as xp,
            tc.tile_pool(name="xt", bufs=1) as xtp,
            tc.tile_pool(name="g", bufs=1) as gp_,
            tc.tile_pool(name="a", bufs=1) as ap_,
            tc.tile_pool(name="ctx", bufs=2) as cxp,
            tc.tile_pool(name="osb", bufs=3) as osp,
            tc.tile_pool(name="ps_g", bufs=1, space="PSUM") as psg,
            tc.tile_pool(name="ps_t", bufs=2, space="PSUM") as pst,
            tc.tile_pool(name="ps_o", bufs=2, space="PSUM") as pso,
        ):
            ident = constp.tile([128, 128], BF, tag="ident")
            make_identity(nc, ident[:])
            # mu*I tiles are created later (after the x1 DMA issues) so the
            # gpsimd stream reaches the first chunk descriptor ASAP.
            muI = constp.tile([128, 128], F32, tag="muI")
            muIb = constp.tile([128, 128], BF, tag="muIb")

            def make_mu_tiles():
                for t_ in (muI, muIb):
                    nc.gpsimd.memset(t_[:], 0.0)
                    nc.gpsimd.affine_select(
                        out=t_[:], in_=t_[:],
                        compare_op=mybir.AluOpType.not_equal, fill=MU,
                        base=0, pattern=[[-1, 128]], channel_multiplier=1,
                    )

            def copy_alt(i, out, in_):
                if i % 2 == 0:
                    nc.scalar.copy(out, in_)
                else:
                    nc.vector.tensor_copy(out, in_)

            xts, gsbs, gtrs, lows, cbds = [], [], [], [], []

            # ---- load x_s, Gram accumulation, and PE transposes ----
            def gram_stage(s, after_dma=None, mid_hook=None):
                gps = []
                for m in range(CB):
                    gt_ = psg.tile([128, 512], F32, tag=f"g{m}",
                                   name=f"gp{m}_{s}")
                    gps.append(gt_)
                xt = xtp.tile([128, CB, N], BF, tag=f"xt{s}")
                xts.append(xt)
                chunks = []
                for r in range(NT // 4):
                    # fp32 DRAM -> bf16 SBUF (SWDGE cast), 1 MiB fp32 chunk.
                    xc = xp.tile([128, 4, C], BF, tag="xc", name=f"xc{s}_{r}")
                    if r == 0 and s == 0:
                        # split the very first chunk so the PE can start on
                        # half the data ~3 us sooner
                        for hh in range(2):
                            src = x_d[s][256 * hh:256 * (hh + 1), :].rearrange(
                                "(t p) c -> p t c", p=128)
                            nc.gpsimd.dma_start(
                                out=xc[:, 2 * hh:2 * (hh + 1), :], in_=src)
                    else:
                        src = x_d[s][512 * r:512 * (r + 1), :].rearrange(
                            "(t p) c -> p t c", p=128)
                        nc.gpsimd.dma_start(out=xc[:], in_=src)
                    chunks.append(xc)
                    if r == 0 and after_dma is not None:
                        after_dma()
                for r in range(NT // 4):
                    xc = chunks[r]
                    for tt_ in range(4):
                        t = 4 * r + tt_
                        for m in range(CB):
                            nc.tensor.matmul(
                                gps[m][:, :C - 128 * m],
                                lhsT=xc[:, tt_, 128 * m:128 * (m + 1)],
                                rhs=xc[:, tt_, 128 * m:],
                                start=(t == 0), stop=(t == NT - 1),
                            )
                        tp4 = pst.tile([128, CB, 128], BF, tag="tps",
                                       name=f"tp4_{s}_{t}")
                        for cb in range(CB):
                            nc.tensor.transpose(
                                tp4[:, cb, :], xc[:, tt_, 128 * cb:128 * (cb + 1)],
                                ident[:])
                        copy_alt(t, xt[:, :, 128 * t:128 * (t + 1)], tp4[:])
                    if r == 1 and mid_hook is not None:
                        mid_hook()

                # Gc = G - mu*I in bf16; gsb[:, a, f] = G[128a+p, f]
                gsb = gp_.tile([128, CB, C], BF, tag=f"g{s}")
                for m in range(CB):
                    nc.vector.tensor_sub(
                        gsb[:, m, 128 * m:128 * (m + 1)],
                        gps[m][:, 0:128], muI[:])
                    if m < CB - 1:
                        copy_alt(m, gsb[:, m, 128 * (m + 1):],
                                 gps[m][:, 128:C - 128 * m])
                # lower-triangle tiles by PE transpose
                low = {}
                gtr = gp_.tile([128, 6, 128], BF, tag=f"gt{s}")
                idx = 0
                for a2 in range(CB):
                    for b2 in range(a2):
                        tp = pst.tile([128, 128], BF, tag="tps")
                        nc.tensor.transpose(
                            tp[:], gsb[:, b2, 128 * a2:128 * (a2 + 1)], ident[:])
                        nc.scalar.copy(gtr[:, idx, :], tp[:])
                        low[(a2, b2)] = idx
                        idx += 1
                gsbs.append(gsb)
                gtrs.append(gtr)
                lows.append(low)

            # ---- weights: load + cast (k-half pre-scaled by SCALE) ----
            whi = []

            wfs = []

            def weight_load(s):
                # SWDGE queue, ordered behind the x chunks so the W bytes
                # never contend with the x-load ramp.
                wf = wfp.tile([128, CB, 2 * C], F32, tag="wf",
                              name=f"wf{s}")
                src = w_d[s][:, :].rearrange("(a p) m -> p a m", p=128)
                nc.gpsimd.dma_start(out=wf[:], in_=src)
                wfs.append(wf)
                wh = wp.tile([128, CB, 2 * C], BF, tag=f"w{s}",
                             name=f"wh{s}")
                whi.append(wh)

            def weight_cast(s):
                # v-half plain cast on scalar, k-half scaled (SCALE = 2^-3,
                # exact in bf16) on vector; scheduled into engine slack.
                wf, wh = wfs[s], whi[s]
                for a in range(CB):
                    nc.scalar.copy(wh[:, a, C:], wf[:, a, C:])
                    nc.vector.tensor_scalar_mul(
                        wh[:, a, 0:C], wf[:, a, 0:C], SCALE)

            # ---- ctx_s: A = G @ Wv (mu folded in), ctp = A^T Wk_scaled ----
            def ctx_mm_stage(s):
                gsb, gtr, low = gsbs[s], gtrs[s], lows[s]

                def g_tile(a2, b2):
                    if b2 >= a2:
                        return gsb[:, a2, 128 * b2:128 * (b2 + 1)]
                    return gtr[:, low[(a2, b2)], :]

                wh = whi[s]
                ahi = ap_.tile([128, CB, C], BF, tag="ah", name=f"ah{s}")
                for b2 in range(CB):
                    apx = pso.tile([128, C], F32, tag="pbig", name=f"apx{s}_{b2}")
                    for a2 in range(CB):
                        nc.tensor.matmul(
                            apx[:], lhsT=g_tile(a2, b2), rhs=wh[:, a2, C:],
                            start=(a2 == 0), stop=False)
                    # += mu * Wv rows of block b2  (G = Gc + mu*I)
                    nc.tensor.matmul(
                        apx[:], lhsT=muIb[:], rhs=wh[:, b2, C:],
                        start=False, stop=True)
                    # bf16 A; halves on scalar & vector in parallel
                    hm = C // 2
                    nc.scalar.copy(ahi[:, b2, 0:hm], apx[:, 0:hm])
                    nc.vector.tensor_copy(ahi[:, b2, hm:], apx[:, hm:])

                # ctp[e, d] per head = (A^T Wk_scaled)
                ctp = pso.tile([64, C], F32, tag="pbig", name=f"ctp{s}")
                for h in range(H):
                    sl = slice(64 * h, 64 * (h + 1))
                    for a2 in range(CB):
                        nc.tensor.matmul(
                            ctp[:, sl], lhsT=ahi[:, a2, sl], rhs=wh[:, a2, sl],
                            start=(a2 == 0), stop=(a2 == CB - 1))
                return ctp

            # ---- softmax over d (free axis) + block-diag ctx tiles ----
            ctxtss = []

            def softmax_pre(s, ctp):
                esb = cxp.tile([64, C], F32, tag="esb")
                ssum = cxp.tile([64, H], F32, tag="ssum")
                rsum = cxp.tile([64, H], F32, tag="rsum")
                nmax = cxp.tile([64, H], F32, tag="nmax")
                ctxts = cxp.tile([64, C], BF, tag="ctxts")
                nc.vector.reduce_max(
                    nmax[:, :], ctp[:, :].rearrange("p (h d) -> p h d", h=H),
                    axis=AX.X, negate=True)
                for h in range(H):
                    sl = slice(64 * h, 64 * (h + 1))
                    nc.scalar.activation(
                        esb[:, sl], ctp[:, sl], AF.Exp,
                        bias=nmax[:, h:h + 1])
                nc.vector.reduce_sum(
                    ssum[:, :], esb[:, :].rearrange("p (h d) -> p h d", h=H),
                    axis=AX.X)
                nc.vector.reciprocal(rsum[:], ssum[:])
                for h in range(H):
                    sl = slice(64 * h, 64 * (h + 1))
                    nc.vector.tensor_scalar_mul(
                        ctxts[:, sl], esb[:, sl], rsum[:, h:h + 1])
                ctxtss.append(ctxts)

            def softmax_cbd(s):
                ctxts = ctxtss[s]
                # 2 heads per PE transpose -> natural ctx block-diag tiles
                cbd = cxp.tile([128, CB, 128], BF, tag=f"cbd{s}")
                nc.vector.memset(cbd[:], 0.0)
                for t2 in range(CB):
                    tp = pst.tile([128, 128], BF, tag="tps")
                    nc.tensor.transpose(
                        tp[:, 0:64], ctxts[:, 128 * t2:128 * (t2 + 1)],
                        ident[0:64, 0:64])
                    nc.scalar.copy(cbd[0:64, t2, 0:64], tp[0:64, 0:64])
                    nc.scalar.copy(cbd[64:128, t2, 64:128], tp[64:128, 0:64])
                cbds.append(cbd)

            # ---- o_s = x_s @ blockdiag(ctx_other), 4-bank rotation ----
            def out_stage(s):
                xt, cbd = xts[s], cbds[1 - s]
                for r in range(NT // 4):
                    ob = osp.tile([128, 4, C], BF, tag="ob")
                    for pp in range(2):       # tile pairs within the group
                        ops = []
                        for q in range(2):
                            t = 4 * r + 2 * pp + q
                            bank = 2 * pp + q
                            op = psg.tile([128, 512], F32, tag=f"g{bank}",
                                          name=f"op{s}_{t}")
                            ops.append(op)
                        for cb in range(CB):  # alternate banks per MM
                            for q in range(2):
                                t = 4 * r + 2 * pp + q
                                nc.tensor.matmul(
                                    ops[q][:, 128 * cb:128 * (cb + 1)],
                                    lhsT=xt[:, cb, 128 * t:128 * (t + 1)],
                                    rhs=cbd[:, cb, :],
                                    start=True, stop=True)
                        for q in range(2):
                            copy_alt(q, ob[:, 2 * pp + q, :], ops[q][:])
                    dst = o_d[s][512 * r:512 * (r + 1), :].rearrange(
                        "(t p) c -> p t c", p=128)
                    nc.sync.dma_start(out=dst, in_=ob[:])

            # SWDGE queue order: x1 chunks, x2 chunk 0, W1, x2 rest, W2.
            gram_stage(0, after_dma=make_mu_tiles)   # x1 load, G1, xT1
            gram_stage(1, after_dma=lambda: weight_load(0),
                       mid_hook=lambda: weight_cast(0))
            weight_load(1)
            weight_cast(1)
            ctp1 = ctx_mm_stage(0)
            softmax_pre(0, ctp1)
            ctp2 = ctx_mm_stage(1)
            softmax_cbd(0)             # -> cbds[0]
            softmax_pre(1, ctp2)       # overlaps out2 below
            out_stage(1)               # o2 = x2 @ Cbd1
            softmax_cbd(1)             # -> cbds[1]
            out_stage(0)               # o1 = x1 @ Cbd2
    nc.compile()
    return nc


_NC = None


def kernel(x1, x2, W_kv1, W_kv2):
    global _NC
    if _NC is None:
        _NC = build()
    x1 = np.ascontiguousarray(x1, dtype=np.float32)
    x2 = np.ascontiguousarray(x2, dtype=np.float32)
    W1 = np.ascontiguousarray(W_kv1, dtype=np.float32)
    W2 = np.ascontiguousarray(W_kv2, dtype=np.float32)
    in_maps = [
        {"x1": x1[b], "x2": x2[b], "W_kv1": W1, "W_kv2": W2} for b in range(B)
    ]
    res = run_bass_kernel_spmd(_NC, in_maps, core_ids=list(range(B)))
    o1 = np.stack([res.results[b]["o1"].astype(np.float32) for b in range(B)])
    o2 = np.stack([res.results[b]["o2"].astype(np.float32) for b in range(B)])
    return o1, o2
